# revision 12
# baseline (speedup 1.0000x reference)
"""BigBird attention (S=4096, D=1024, H=16, window=256, 1 global, 32 random)
on 8 TRN2 NeuronCores, head-sharded (2 heads / core).

Layout strategy (per core c, heads 2c..2c+1, feature slice F = c*128:(c+1)*128):
  - qT/kT kept transposed [head_dim(part), seq]  -> scores computed directly as
    scoresT[k, q] = kT.T @ qT in float32r (no PE transposes in the hot loop)
  - exp on ScalarE psum->sbuf bf16 (no max subtraction; scores ~ N(0,1))
  - multiplicative bf16 mask (1/0), applied on VectorE at 2x mode
  - attn@v with a ones-column appended to V: softmax denominators fall out of
    the same matmul; normalization via PE-broadcast + fast reciprocal
  - projections and out-proj run in bf16 (inputs bf16 from host)
  - out-proj partial = attn_norm.T @ WoT_c ; host sums the 8 partials
    (= the all-reduce) and adds the constant bv @ Wo.T + bo term
    (exact: softmax rows sum to 1, so the v-bias passes through attention).
"""

import os
from contextlib import ExitStack

import numpy as np
import ml_dtypes

import concourse.bass as bass
import concourse.bacc as bacc
import concourse.tile as tile
from concourse import mybir
import concourse.bass_utils as bass_utils

# no artifact bucket in this container; only used when tracing
bass_utils.upload_artifacts = lambda tmpdir: "local"

F32 = mybir.dt.float32
F32R = mybir.dt.float32r
BF16 = mybir.dt.bfloat16
AFT = mybir.ActivationFunctionType

S = 4096          # sequence length
D = 1024          # d_model
F = 128           # features per core (2 heads x 64)
DH = 64           # head dim
NCORES = 8
KF = 8            # contraction chunks of 128 over D
SC = 8            # seq chunks of 512 (projections)
KT = 32           # key tiles of 128
QB = 8            # query blocks of 512
VW = 2 * (DH + 1)  # per-ktile v_aug width (65 per head)

LAST_EXEC_TIME_NS = None
LAST_RESULT = None

_NC_CACHE = None


def _build_nc():
    nc = bacc.Bacc("TRN2", target_bir_lowering=False, debug=False)

    xT = nc.declare_dram_parameter("xT", [D, S], BF16, isOutput=False)
    WqT = nc.declare_dram_parameter("WqT", [D, F], BF16, isOutput=False)
    WkT = nc.declare_dram_parameter("WkT", [D, F], BF16, isOutput=False)
    WvT = nc.declare_dram_parameter("WvT", [D, F], BF16, isOutput=False)
    bqv = nc.declare_dram_parameter("bqv", [F, 1], F32, isOutput=False)
    bkv = nc.declare_dram_parameter("bkv", [F, 1], F32, isOutput=False)
    WoT = nc.declare_dram_parameter("WoT", [F, D], BF16, isOutput=False)
    maskT = nc.declare_dram_parameter("maskT", [S, S], BF16, isOutput=False)
    ident = nc.declare_dram_parameter("ident", [128, 128], BF16, isOutput=False)
    onesv = nc.declare_dram_parameter("onesv", [1, DH], BF16, isOutput=False)
    out = nc.declare_dram_parameter("out", [S, D], F32, isOutput=True)

    with tile.TileContext(nc) as tc:
        with ExitStack() as ctx:
            # ---- persistent sbuf ----
            wpool = ctx.enter_context(tc.tile_pool(name="w", bufs=1))
            wq = wpool.tile([128, D], BF16, tag="wq")
            wk = wpool.tile([128, D], BF16, tag="wk")
            wv = wpool.tile([128, D], BF16, tag="wv")
            for kf in range(KF):
                sl = slice(kf * 128, (kf + 1) * 128)
                nc.sync.dma_start(wq[:, sl], WqT[sl, :])
                nc.sync.dma_start(wk[:, sl], WkT[sl, :])
                nc.sync.dma_start(wv[:, sl], WvT[sl, :])
            wo = wpool.tile([128, D], BF16, tag="wo")
            nc.sync.dma_start(wo[:], WoT[:, :])
            bq_sb = wpool.tile([128, 1], F32, tag="bq")
            bk_sb = wpool.tile([128, 1], F32, tag="bk")
            nc.sync.dma_start(bq_sb[:], bqv[:, :])
            nc.sync.dma_start(bk_sb[:], bkv[:, :])
            id_sb = wpool.tile([128, 128], BF16, tag="id")
            nc.sync.dma_start(id_sb[:], ident[:, :])
            ones_sb = wpool.tile([1, DH], BF16, tag="ones")
            nc.sync.dma_start(ones_sb[:], onesv[:, :])

            # per-chunk tiles so phase C can start as soon as a chunk is ready
            qTs = [wpool.tile([128, 512], BF16, tag=f"qT{i}", name=f"qT{i}")
                   for i in range(SC)]
            kTs = [wpool.tile([128, 512], BF16, tag=f"kT{i}", name=f"kT{i}")
                   for i in range(SC)]
            vs = [wpool.tile([128, VW], BF16, tag=f"v{t}", name=f"v{t}")
                  for t in range(KT)]
            for t in range(KT):
                nc.vector.memset(vs[t][:], 1.0)  # ones cols survive at 64/129

            def kt_ap(h, t):  # [64, 128] head h, key tile t
                return kTs[t // 4][h * DH:(h + 1) * DH,
                                   (t % 4) * 128:(t % 4 + 1) * 128]

            def qt_ap(h, qb):  # [64, 512] head h, query block qb
                return qTs[qb][h * DH:(h + 1) * DH, :]

            # ---- phase B: projections ----
            with (
                tc.tile_pool(name="xs", bufs=6) as xpool,
                tc.tile_pool(name="vt", bufs=2) as vtpool,
                tc.tile_pool(name="prps", bufs=2, space="PSUM") as prpool,
                tc.tile_pool(name="tpps", bufs=2, space="PSUM") as tppool,
            ):
                for sc in range(SC):
                    ssl = slice(sc * 512, (sc + 1) * 512)
                    psq = prpool.tile([128, 512], F32, tag="psq")
                    psk = prpool.tile([128, 512], F32, tag="psk")
                    psv = prpool.tile([128, 512], F32, tag="psv")
                    for kf in range(KF):
                        ksl = slice(kf * 128, (kf + 1) * 128)
                        xt = xpool.tile([128, 512], BF16)
                        nc.sync.dma_start(xt[:], xT[ksl, ssl])
                        st, sp = kf == 0, kf == KF - 1
                        nc.tensor.matmul(psq[:], wq[:, ksl], xt[:],
                                         start=st, stop=sp)
                        nc.tensor.matmul(psk[:], wk[:, ksl], xt[:],
                                         start=st, stop=sp)
                        nc.tensor.matmul(psv[:], wv[:, ksl], xt[:],
                                         start=st, stop=sp)
                    nc.scalar.activation(qTs[sc][:], psq[:], AFT.Identity,
                                         bias=bq_sb[:], scale=1.0)
                    nc.scalar.activation(kTs[sc][:], psk[:], AFT.Identity,
                                         bias=bk_sb[:], scale=1.0)
                    vt = vtpool.tile([128, 512], BF16)
                    nc.scalar.activation(vt[:], psv[:], AFT.Identity)
                    # transpose v into natural [seq, feat] layout -> v_aug
                    for j in range(4):
                        t_g = sc * 4 + j
                        pst = tppool.tile([128, 128], BF16, tag="pst")
                        nc.tensor.transpose(pst[:], vt[:, j * 128:(j + 1) * 128],
                                            id_sb[:])
                        dst = vs[t_g][:].rearrange("p (h x) -> p h x",
                                                   h=2)[:, :, 0:DH]
                        src = pst[:].rearrange("p (h x) -> p h x", h=2)
                        nc.vector.tensor_copy(dst, src)

            # ---- phase C: attention + out-proj ----
            with (
                tc.tile_pool(name="mask", bufs=10) as mpool,
                tc.tile_pool(name="attn", bufs=8) as atpool,
                tc.tile_pool(name="an", bufs=2) as anpool,
                tc.tile_pool(name="dn", bufs=2) as dpool,
                tc.tile_pool(name="rc", bufs=2) as rpool,
                tc.tile_pool(name="og", bufs=3) as ogpool,
                tc.tile_pool(name="sps", bufs=2, space="PSUM") as spool,
                tc.tile_pool(name="ops", bufs=2, space="PSUM") as opool,
                tc.tile_pool(name="pps", bufs=2, space="PSUM") as ppool,
            ):
                for qb in range(QB):
                    qsl = slice(qb * 512, (qb + 1) * 512)
                    ps_o = [opool.tile([DH + 1, 512], F32, tag="ps_o",
                                       name=f"ps_o_{qb}_{h}")
                            for h in range(2)]
                    for t in range(KT):
                        tsl = slice(t * 128, (t + 1) * 128)
                        msk = mpool.tile([128, 512], BF16)
                        nc.sync.dma_start(msk[:], maskT[tsl, qsl])
                        ps_s = spool.tile([128, 1024], F32, tag="ps_s")
                        for h in range(2):
                            nc.tensor.matmul(
                                ps_s[:, h * 512:(h + 1) * 512],
                                kt_ap(h, t), qt_ap(h, qb),
                                start=True, stop=True,
                                tile_position=(h * DH, 0))
                        at = atpool.tile([128, 1024], BF16)
                        nc.scalar.activation(at[:], ps_s[:], AFT.Exp)
                        for h in range(2):
                            asl = slice(h * 512, (h + 1) * 512)
                            nc.vector.tensor_mul(at[:, asl], at[:, asl], msk[:])
                        for h in range(2):
                            nc.tensor.matmul(
                                ps_o[h][:],
                                vs[t][:, h * (DH + 1):(h + 1) * (DH + 1)],
                                at[:, h * 512:(h + 1) * 512],
                                start=(t == 0), stop=(t == KT - 1))
                    # normalize
                    an = anpool.tile([128, 512], BF16)
                    for h in range(2):
                        dn = dpool.tile([1, 512], BF16)
                        nc.vector.tensor_copy(dn[:], ps_o[h][DH:DH + 1, :])
                        ps_b = ppool.tile([DH, 512], F32, tag="ps_p")
                        nc.tensor.matmul(ps_b[:], ones_sb[:], dn[:],
                                         start=True, stop=True)
                        rc = rpool.tile([DH, 512], F32)
                        nc.vector.reciprocal_approx_fast(rc[:], ps_b[:])
                        nc.vector.tensor_mul(an[h * DH:(h + 1) * DH, :],
                                             ps_o[h][0:DH, :], rc[:])
                    # out-proj partial for this q block
                    for stt in range(4):
                        for oc in range(2):
                            po = ppool.tile([128, 512], F32, tag="ps_p")
                            nc.tensor.matmul(
                                po[:],
                                an[:, stt * 128:(stt + 1) * 128],
                                wo[:, oc * 512:(oc + 1) * 512],
                                start=True, stop=True)
                            og = ogpool.tile([128, 512], F32)
                            nc.vector.tensor_copy(og[:], po[:])
                            r0 = qb * 512 + stt * 128
                            nc.sync.dma_start(
                                out[r0:r0 + 128, oc * 512:(oc + 1) * 512], og[:])

    nc.compile()
    return nc


def _get_nc():
    global _NC_CACHE
    if _NC_CACHE is None:
        _NC_CACHE = _build_nc()
    return _NC_CACHE


def kernel(x, Wq, bq, Wk, bk, Wv, bv, Wo, bo, mask):
    global LAST_EXEC_TIME_NS, LAST_RESULT
    x = np.asarray(x, dtype=np.float32).reshape(S, D)
    Wq = np.asarray(Wq, dtype=np.float32)
    Wk = np.asarray(Wk, dtype=np.float32)
    Wv = np.asarray(Wv, dtype=np.float32)
    Wo = np.asarray(Wo, dtype=np.float32)
    bq = np.asarray(bq, dtype=np.float32)
    bk = np.asarray(bk, dtype=np.float32)
    bv = np.asarray(bv, dtype=np.float32)
    bo = np.asarray(bo, dtype=np.float32)
    mask = np.asarray(mask, dtype=np.float32)

    scale = DH ** -0.5
    xTb = np.ascontiguousarray(x.T.astype(ml_dtypes.bfloat16))    # [D, S]
    maskT_m = np.ascontiguousarray(
        (mask == 0).T.astype(ml_dtypes.bfloat16))                 # [k, q] 1/0
    ident = np.eye(128, dtype=ml_dtypes.bfloat16)

    in_maps = []
    for c in range(NCORES):
        sl = slice(c * F, (c + 1) * F)
        in_maps.append({
            "xT": xTb,
            "WqT": np.ascontiguousarray(
                (Wq[sl, :] * scale).T.astype(ml_dtypes.bfloat16)),
            "WkT": np.ascontiguousarray(Wk[sl, :].T.astype(ml_dtypes.bfloat16)),
            "WvT": np.ascontiguousarray(Wv[sl, :].T.astype(ml_dtypes.bfloat16)),
            "bqv": np.ascontiguousarray((bq[sl] * scale).reshape(F, 1)),
            "bkv": np.ascontiguousarray(bk[sl].reshape(F, 1)),
            "WoT": np.ascontiguousarray(Wo[:, sl].T.astype(ml_dtypes.bfloat16)),
            "maskT": maskT_m,
            "ident": ident,
            "onesv": np.ones((1, DH), dtype=ml_dtypes.bfloat16),
        })

    nc = _get_nc()
    res = bass_utils.run_bass_kernel_spmd(
        nc, in_maps, core_ids=list(range(NCORES)))
    LAST_EXEC_TIME_NS = res.exec_time_ns
    LAST_RESULT = res

    acc = np.zeros((S, D), dtype=np.float32)
    for c in range(NCORES):
        acc += res.results[c]["out"]
    acc += bv @ Wo.T + bo
    return acc.reshape(1, S, D)


# revision 13
# speedup vs baseline: 1.0412x; 1.0412x over previous
"""BigBird attention (S=4096, D=1024, H=16, window=256, 1 global, 32 random)
on 8 TRN2 NeuronCores, head-sharded (2 heads / core).

Layout strategy (per core c, heads 2c..2c+1, feature slice F = c*128:(c+1)*128):
  - qT/kT kept transposed [head_dim(part), seq]  -> scores computed directly as
    scoresT[k, q] = kT.T @ qT in float32r (no PE transposes in the hot loop)
  - exp on ScalarE psum->sbuf bf16 (no max subtraction; scores ~ N(0,1))
  - multiplicative bf16 mask (1/0), applied on VectorE at 2x mode
  - attn@v with a ones-column appended to V: softmax denominators fall out of
    the same matmul; normalization via PE-broadcast + fast reciprocal
  - projections and out-proj run in bf16 (inputs bf16 from host)
  - out-proj partial = attn_norm.T @ WoT_c ; host sums the 8 partials
    (= the all-reduce) and adds the constant bv @ Wo.T + bo term
    (exact: softmax rows sum to 1, so the v-bias passes through attention).
"""

import os
from contextlib import ExitStack

import numpy as np
import ml_dtypes

import concourse.bass as bass
import concourse.bacc as bacc
import concourse.tile as tile
from concourse import mybir
import concourse.bass_utils as bass_utils

# no artifact bucket in this container; only used when tracing
bass_utils.upload_artifacts = lambda tmpdir: "local"

F32 = mybir.dt.float32
F32R = mybir.dt.float32r
BF16 = mybir.dt.bfloat16
AFT = mybir.ActivationFunctionType

S = 4096          # sequence length
D = 1024          # d_model
F = 128           # features per core (2 heads x 64)
DH = 64           # head dim
NCORES = 8
KF = 8            # contraction chunks of 128 over D
SC = 8            # seq chunks of 512 (projections)
KT = 32           # key tiles of 128
QB = 8            # query blocks of 512
VW = 2 * (DH + 1)  # per-ktile v_aug width (65 per head)

LAST_EXEC_TIME_NS = None
LAST_RESULT = None

_NC_CACHE = None


def _build_nc():
    nc = bacc.Bacc("TRN2", target_bir_lowering=False, debug=False)

    xT = nc.declare_dram_parameter("xT", [D, S], BF16, isOutput=False)
    WqT = nc.declare_dram_parameter("WqT", [D, F], BF16, isOutput=False)
    WkT = nc.declare_dram_parameter("WkT", [D, F], BF16, isOutput=False)
    WvT = nc.declare_dram_parameter("WvT", [D, F], BF16, isOutput=False)
    bqv = nc.declare_dram_parameter("bqv", [F, 1], F32, isOutput=False)
    bkv = nc.declare_dram_parameter("bkv", [F, 1], F32, isOutput=False)
    WoT = nc.declare_dram_parameter("WoT", [F, D], BF16, isOutput=False)
    maskT = nc.declare_dram_parameter("maskT", [S, S], BF16, isOutput=False)
    ident = nc.declare_dram_parameter("ident", [128, 128], BF16, isOutput=False)
    onesv = nc.declare_dram_parameter("onesv", [1, DH], BF16, isOutput=False)
    out = nc.declare_dram_parameter("out", [S, D], F32, isOutput=True)

    with tile.TileContext(nc) as tc:
        with ExitStack() as ctx:
            # ---- persistent sbuf ----
            wpool = ctx.enter_context(tc.tile_pool(name="w", bufs=1))
            wq = wpool.tile([128, D], BF16, tag="wq")
            wk = wpool.tile([128, D], BF16, tag="wk")
            wv = wpool.tile([128, D], BF16, tag="wv")
            for kf in range(KF):
                sl = slice(kf * 128, (kf + 1) * 128)
                nc.sync.dma_start(wq[:, sl], WqT[sl, :])
                nc.sync.dma_start(wk[:, sl], WkT[sl, :])
                nc.sync.dma_start(wv[:, sl], WvT[sl, :])
            wo = wpool.tile([128, D], BF16, tag="wo")
            nc.sync.dma_start(wo[:], WoT[:, :])
            bq_sb = wpool.tile([128, 1], F32, tag="bq")
            bk_sb = wpool.tile([128, 1], F32, tag="bk")
            nc.sync.dma_start(bq_sb[:], bqv[:, :])
            nc.sync.dma_start(bk_sb[:], bkv[:, :])
            id_sb = wpool.tile([128, 128], BF16, tag="id")
            nc.sync.dma_start(id_sb[:], ident[:, :])
            ones_sb = wpool.tile([1, DH], BF16, tag="ones")
            nc.sync.dma_start(ones_sb[:], onesv[:, :])

            # per-chunk tiles so phase C can start as soon as a chunk is ready
            qTs = [wpool.tile([128, 512], BF16, tag=f"qT{i}", name=f"qT{i}")
                   for i in range(SC)]
            kTs = [wpool.tile([128, 512], BF16, tag=f"kT{i}", name=f"kT{i}")
                   for i in range(SC)]
            vs = [wpool.tile([128, VW], BF16, tag=f"v{t}", name=f"v{t}")
                  for t in range(KT)]
            for t in range(KT):
                nc.vector.memset(vs[t][:], 1.0)  # ones cols survive at 64/129

            def kt_ap(h, t):  # [64, 128] head h, key tile t
                return kTs[t // 4][h * DH:(h + 1) * DH,
                                   (t % 4) * 128:(t % 4 + 1) * 128]

            def qt_ap(h, qb):  # [64, 512] head h, query block qb
                return qTs[qb][h * DH:(h + 1) * DH, :]

            # ---- phase B: projections ----
            with (
                tc.tile_pool(name="xs", bufs=6) as xpool,
                tc.tile_pool(name="vt", bufs=2) as vtpool,
                tc.tile_pool(name="prps", bufs=2, space="PSUM") as prpool,
                tc.tile_pool(name="tpps", bufs=2, space="PSUM") as tppool,
            ):
                for sc in range(SC):
                    ssl = slice(sc * 512, (sc + 1) * 512)
                    psq = prpool.tile([128, 512], F32, tag="psq")
                    psk = prpool.tile([128, 512], F32, tag="psk")
                    psv = prpool.tile([128, 512], F32, tag="psv")
                    for kf in range(KF):
                        ksl = slice(kf * 128, (kf + 1) * 128)
                        xt = xpool.tile([128, 512], BF16)
                        nc.sync.dma_start(xt[:], xT[ksl, ssl])
                        st, sp = kf == 0, kf == KF - 1
                        nc.tensor.matmul(psq[:], wq[:, ksl], xt[:],
                                         start=st, stop=sp)
                        nc.tensor.matmul(psk[:], wk[:, ksl], xt[:],
                                         start=st, stop=sp)
                        nc.tensor.matmul(psv[:], wv[:, ksl], xt[:],
                                         start=st, stop=sp)
                    nc.scalar.activation(qTs[sc][:], psq[:], AFT.Identity,
                                         bias=bq_sb[:], scale=1.0)
                    nc.scalar.activation(kTs[sc][:], psk[:], AFT.Identity,
                                         bias=bk_sb[:], scale=1.0)
                    vt = vtpool.tile([128, 512], BF16)
                    nc.scalar.activation(vt[:], psv[:], AFT.Identity)
                    # transpose v into natural [seq, feat] layout -> v_aug
                    for j in range(4):
                        t_g = sc * 4 + j
                        pst = tppool.tile([128, 128], BF16, tag="pst")
                        nc.tensor.transpose(pst[:], vt[:, j * 128:(j + 1) * 128],
                                            id_sb[:])
                        dst = vs[t_g][:].rearrange("p (h x) -> p h x",
                                                   h=2)[:, :, 0:DH]
                        src = pst[:].rearrange("p (h x) -> p h x", h=2)
                        nc.vector.tensor_copy(dst, src)

            # ---- phase C: attention + out-proj ----
            with (
                tc.tile_pool(name="mask", bufs=10) as mpool,
                tc.tile_pool(name="attn", bufs=8) as atpool,
                tc.tile_pool(name="an", bufs=2) as anpool,
                tc.tile_pool(name="dn", bufs=2) as dpool,
                tc.tile_pool(name="rc", bufs=2) as rpool,
                tc.tile_pool(name="og", bufs=3) as ogpool,
                tc.tile_pool(name="sps", bufs=3, space="PSUM") as spool,
                tc.tile_pool(name="ops", bufs=2, space="PSUM") as opool,
            ):
                for qb in range(QB):
                    qsl = slice(qb * 512, (qb + 1) * 512)
                    ps_o = [opool.tile([DH + 1, 512], F32, tag="ps_o",
                                       name=f"ps_o_{qb}_{h}")
                            for h in range(2)]
                    for t in range(KT):
                        tsl = slice(t * 128, (t + 1) * 128)
                        msk = mpool.tile([128, 512], BF16)
                        nc.sync.dma_start(msk[:], maskT[tsl, qsl])
                        ps_s = spool.tile([128, 1024], F32, tag="ps_s")
                        for h in range(2):
                            nc.tensor.matmul(
                                ps_s[:, h * 512:(h + 1) * 512],
                                kt_ap(h, t), qt_ap(h, qb),
                                start=True, stop=True,
                                tile_position=(h * DH, 0))
                        at = atpool.tile([128, 1024], BF16)
                        nc.scalar.activation(at[:], ps_s[:], AFT.Exp)
                        for h in range(2):
                            asl = slice(h * 512, (h + 1) * 512)
                            nc.vector.tensor_mul(at[:, asl], at[:, asl], msk[:])
                        for h in range(2):
                            nc.tensor.matmul(
                                ps_o[h][:],
                                vs[t][:, h * (DH + 1):(h + 1) * (DH + 1)],
                                at[:, h * 512:(h + 1) * 512],
                                start=(t == 0), stop=(t == KT - 1))
                    # normalize
                    an = anpool.tile([128, 512], BF16)
                    for h in range(2):
                        dn = dpool.tile([1, 512], BF16)
                        nc.vector.tensor_copy(dn[:], ps_o[h][DH:DH + 1, :])
                        ps_b = spool.tile([DH, 512], F32, tag="ps_s",
                                          name=f"ps_b_{qb}_{h}")
                        nc.tensor.matmul(ps_b[:], ones_sb[:], dn[:],
                                         start=True, stop=True)
                        rc = rpool.tile([DH, 512], F32)
                        nc.vector.reciprocal_approx_fast(rc[:], ps_b[:])
                        nc.vector.tensor_mul(an[h * DH:(h + 1) * DH, :],
                                             ps_o[h][0:DH, :], rc[:])
                    # out-proj partial for this q block
                    for stt in range(4):
                        for oc in range(2):
                            po = spool.tile([128, 512], F32, tag="ps_s",
                                            name=f"po_{qb}_{stt}_{oc}")
                            nc.tensor.matmul(
                                po[:],
                                an[:, stt * 128:(stt + 1) * 128],
                                wo[:, oc * 512:(oc + 1) * 512],
                                start=True, stop=True)
                            og = ogpool.tile([128, 512], F32)
                            nc.vector.tensor_copy(og[:], po[:])
                            r0 = qb * 512 + stt * 128
                            nc.sync.dma_start(
                                out[r0:r0 + 128, oc * 512:(oc + 1) * 512], og[:])

    nc.compile()
    return nc


def _get_nc():
    global _NC_CACHE
    if _NC_CACHE is None:
        _NC_CACHE = _build_nc()
    return _NC_CACHE


def kernel(x, Wq, bq, Wk, bk, Wv, bv, Wo, bo, mask):
    global LAST_EXEC_TIME_NS, LAST_RESULT
    x = np.asarray(x, dtype=np.float32).reshape(S, D)
    Wq = np.asarray(Wq, dtype=np.float32)
    Wk = np.asarray(Wk, dtype=np.float32)
    Wv = np.asarray(Wv, dtype=np.float32)
    Wo = np.asarray(Wo, dtype=np.float32)
    bq = np.asarray(bq, dtype=np.float32)
    bk = np.asarray(bk, dtype=np.float32)
    bv = np.asarray(bv, dtype=np.float32)
    bo = np.asarray(bo, dtype=np.float32)
    mask = np.asarray(mask, dtype=np.float32)

    scale = DH ** -0.5
    xTb = np.ascontiguousarray(x.T.astype(ml_dtypes.bfloat16))    # [D, S]
    maskT_m = np.ascontiguousarray(
        (mask == 0).T.astype(ml_dtypes.bfloat16))                 # [k, q] 1/0
    ident = np.eye(128, dtype=ml_dtypes.bfloat16)

    in_maps = []
    for c in range(NCORES):
        sl = slice(c * F, (c + 1) * F)
        in_maps.append({
            "xT": xTb,
            "WqT": np.ascontiguousarray(
                (Wq[sl, :] * scale).T.astype(ml_dtypes.bfloat16)),
            "WkT": np.ascontiguousarray(Wk[sl, :].T.astype(ml_dtypes.bfloat16)),
            "WvT": np.ascontiguousarray(Wv[sl, :].T.astype(ml_dtypes.bfloat16)),
            "bqv": np.ascontiguousarray((bq[sl] * scale).reshape(F, 1)),
            "bkv": np.ascontiguousarray(bk[sl].reshape(F, 1)),
            "WoT": np.ascontiguousarray(Wo[:, sl].T.astype(ml_dtypes.bfloat16)),
            "maskT": maskT_m,
            "ident": ident,
            "onesv": np.ones((1, DH), dtype=ml_dtypes.bfloat16),
        })

    nc = _get_nc()
    res = bass_utils.run_bass_kernel_spmd(
        nc, in_maps, core_ids=list(range(NCORES)))
    LAST_EXEC_TIME_NS = res.exec_time_ns
    LAST_RESULT = res

    acc = np.zeros((S, D), dtype=np.float32)
    for c in range(NCORES):
        acc += res.results[c]["out"]
    acc += bv @ Wo.T + bo
    return acc.reshape(1, S, D)
